# revision 1
# baseline (speedup 1.0000x reference)
"""Causal self-attention (GQA, RoPE) Trainium2 Bass kernel.

Full inputs in, full output out. Tensor-parallel over heads across 8
NeuronCores: core i computes q-heads 4i..4i+3 (kv head i) and a partial
output projection over its 256 attn-out features; the host sums the 8
partial outputs (the "all-reduce after output_proj" step).
"""

import numpy as np

import concourse.bacc as bacc
import concourse.mybir as mybir
import concourse.tile as tile
from concourse.bass_utils import run_bass_kernel_spmd

S = 2048          # sequence length
E = 2048          # embedding dim
H = 32            # query heads
KV = 8            # kv heads
HD = 64           # head dim
NCORES = 8
HC = H // NCORES  # query heads per core = 4
DQ = HC * HD      # per-core q proj width = 256
DKV = HD          # per-core kv proj width = 64
DQK = DQ + DKV    # roped span = 320
DW = DQ + 2 * DKV  # fused qkv proj width = 384
ST = S // 128     # 16 s-tiles of 128 rows
MASK_NEG = -1.0e4  # pre-scale additive mask (scaled: -1250 -> exp == 0)

F32 = mybir.dt.float32
F32R = mybir.dt.float32r


def r(ap):
    """Bitcast an AP to float32r so the PE runs fast-mode fp32 matmuls."""
    return ap.bitcast(F32R)


def build_nc(seq_tiles=ST, reps=1, phases=(1, 2, 3)):
    """Build + compile the per-core Bass program (identical on all cores)."""
    st_n = seq_tiles
    s_n = st_n * 128
    qb_n = s_n // 512

    nc = bacc.Bacc("TRN2", target_bir_lowering=False, debug=False)
    x_d = nc.dram_tensor("x", [s_n, E], F32R, kind="ExternalInput")
    wt_d = nc.dram_tensor("wt", [E, DW], F32R, kind="ExternalInput")
    wot_d = nc.dram_tensor("wot", [DQ, E], F32R, kind="ExternalInput")
    cos_d = nc.dram_tensor("cosh", [s_n, DQK // 2], F32, kind="ExternalInput")
    sin_d = nc.dram_tensor("sinh", [s_n, DQK // 2], F32, kind="ExternalInput")
    mask_d = nc.dram_tensor("maskadd", [512, 512], F32, kind="ExternalInput")
    id_d = nc.dram_tensor("ident", [128, 128], F32R, kind="ExternalInput")
    out_d = nc.dram_tensor("out", [s_n, E], F32, kind="ExternalOutput")

    with tile.TileContext(nc) as tc:
        for _rep in range(reps):
            # ---------- persistent constants / cross-phase tensors ----------
            with (
                tc.tile_pool(name="const", bufs=1) as constp,
                tc.tile_pool(name="qkv_store", bufs=1) as storep,
            ):
                ident = constp.tile([128, 128], F32R)
                nc.sync.dma_start(out=ident[:], in_=id_d.ap()[:, :])

                woT_sb = constp.tile([128, 2, E], F32R)
                nc.sync.dma_start(
                    out=woT_sb[:], in_=wot_d.ap().rearrange("(c p) e -> p c e", p=128)
                )
                mask_sb = constp.tile([128, 4, 512], F32)
                nc.sync.dma_start(
                    out=mask_sb[:], in_=mask_d.ap().rearrange("(r p) k -> p r k", p=128)
                )

                # qT: all heads on partitions 0:64; head h of s-tile t in
                # cols t*512 + h*128.
                qT_sb = storep.tile([64, st_n * 512], F32R)
                # kT: kv head on partitions 0:64.
                kT_sb = storep.tile([64, s_n], F32R)
                # v: [s, d] duplicated along free (cols 2*64 per s-tile) so the
                # AV matmul writes the full 128 psum partitions.
                v_sb = storep.tile([128, st_n * 2 * DKV], F32R)
                # attn-out transposed: head-pair hp in col block hp*s_n.
                aoT_sb = storep.tile([128, 2 * s_n], F32R)

                # ================= phase 1: qkv proj + rope =================
                with (
                    tc.tile_pool(name="p1_sbuf", bufs=2) as p1,
                    tc.tile_pool(name="p1_w", bufs=1) as p1w,
                    tc.tile_pool(name="p1_xt", bufs=3) as p1x,
                    tc.tile_pool(name="p1_ps_xt", bufs=2, space="PSUM") as ps_xt_p,
                    tc.tile_pool(name="p1_ps_qkv", bufs=2, space="PSUM") as ps_qkv_p,
                    tc.tile_pool(name="p1_ps_tr", bufs=2, space="PSUM") as ps_tr_p,
                ):
                    wT_sb = p1w.tile([128, E // 128, DW], F32R)
                    for j in range(E // 128):
                        nc.sync.dma_start(
                            out=wT_sb[:, j, :],
                            in_=wt_d.ap()[j * 128:(j + 1) * 128, :],
                        )

                    for t in range(st_n if 1 in phases else 0):
                        x_sb = p1.tile([128, E], F32R, tag="x")
                        nc.sync.dma_start(
                            out=x_sb[:], in_=x_d.ap()[t * 128:(t + 1) * 128, :]
                        )
                        cs_sb = p1.tile([128, 2, DQK // 2], F32, tag="cs")
                        nc.sync.dma_start(
                            out=cs_sb[:, 0, :], in_=cos_d.ap()[t * 128:(t + 1) * 128, :]
                        )
                        nc.sync.dma_start(
                            out=cs_sb[:, 1, :], in_=sin_d.ap()[t * 128:(t + 1) * 128, :]
                        )
                        ps_qkv = ps_qkv_p.tile([128, DW], F32, tag="qkv")
                        for jg in range(E // 512):
                            ps_xt = ps_xt_p.tile([128, 512], F32, tag="xt")
                            for m in range(4):
                                nc.tensor.matmul(
                                    r(ps_xt[:, m * 128:(m + 1) * 128]),
                                    r(x_sb[:, (4 * jg + m) * 128:(4 * jg + m + 1) * 128]),
                                    r(ident[:]),
                                    is_transpose=True,
                                    start=(m == 0),
                                    stop=(m == 3),
                                )
                            xt_sb = p1x.tile([128, 512], F32R, tag="xts")
                            nc.vector.tensor_copy(xt_sb[:], ps_xt[:])
                            for m in range(4):
                                j = 4 * jg + m
                                nc.tensor.matmul(
                                    ps_qkv[:],
                                    r(xt_sb[:, m * 128:(m + 1) * 128]),
                                    r(wT_sb[:, j, :]),
                                    start=(j == 0),
                                    stop=(j == E // 128 - 1),
                                )

                        # ---- rope on q+k jointly (320 cols); copy v ----
                        pairs = DQK // 2  # 160
                        qk_sb = p1.tile([128, DQK], F32R, tag="qkro")
                        se = ps_qkv[:, 0:DQK].rearrange("p (n two) -> p two n", two=2)
                        de = qk_sb[:].rearrange("p (n two) -> p two n", two=2)
                        c_ap = cs_sb[:, 0, :]
                        s_ap = cs_sb[:, 1, :]
                        t1 = p1.tile([128, pairs], F32, tag="t1")
                        t2 = p1.tile([128, pairs], F32, tag="t2")
                        nc.vector.tensor_mul(t1[:], se[:, 0, :], c_ap)
                        nc.vector.tensor_mul(t2[:], se[:, 1, :], s_ap)
                        nc.vector.tensor_sub(de[:, 0, :], t1[:], t2[:])
                        t3 = p1.tile([128, pairs], F32, tag="t3")
                        t4 = p1.tile([128, pairs], F32, tag="t4")
                        nc.vector.tensor_mul(t3[:], se[:, 1, :], c_ap)
                        nc.vector.tensor_mul(t4[:], se[:, 0, :], s_ap)
                        nc.vector.tensor_add(de[:, 1, :], t3[:], t4[:])

                        for dup in range(2):
                            nc.vector.tensor_copy(
                                v_sb[:, t * 2 * DKV + dup * DKV:t * 2 * DKV + (dup + 1) * DKV],
                                ps_qkv[:, DQK:DW],
                            )

                        # ---- transpose roped q/k into qT/kT (partitions 0:64) ----
                        ps_trq = ps_tr_p.tile([64, 512], F32, tag="trq")
                        for hh in range(4):
                            nc.tensor.matmul(
                                r(ps_trq[:, hh * 128:(hh + 1) * 128]),
                                r(qk_sb[:, hh * 64:(hh + 1) * 64]),
                                r(ident[:]),
                                is_transpose=True,
                                start=(hh == 0),
                                stop=(hh == 3),
                            )
                        nc.vector.tensor_copy(
                            qT_sb[:, t * 512:(t + 1) * 512], ps_trq[:]
                        )
                        ps_trk = ps_tr_p.tile([64, 128], F32, tag="trk")
                        nc.tensor.matmul(
                            r(ps_trk[:]), r(qk_sb[:, 256:DQK]), r(ident[:]),
                            is_transpose=True, start=True, stop=True,
                        )
                        nc.vector.tensor_copy(
                            kT_sb[:, t * 128:(t + 1) * 128], ps_trk[:]
                        )

                # ================= phase 2: attention =================
                with (
                    tc.tile_pool(name="p2_a", bufs=3) as p2a,
                    tc.tile_pool(name="p2_at", bufs=2) as p2t,
                    tc.tile_pool(name="p2_small", bufs=12) as p2s,
                    tc.tile_pool(name="p3_o", bufs=2) as p3o,
                    tc.tile_pool(name="p2_ps_s", bufs=3, space="PSUM") as ps_s_p,
                    tc.tile_pool(name="p2_ps_at", bufs=3, space="PSUM") as ps_at_p,
                    tc.tile_pool(name="p2_ps_av", bufs=1, space="PSUM") as ps_av_p,
                    tc.tile_pool(name="p3_ps", bufs=1, space="PSUM") as ps_o_p,
                ):
                    for qb in range(qb_n if 2 in phases else 0):
                        for h in range(HC):
                            p0 = 64 * (h & 1)
                            hp2 = h >> 1
                            nch = 4 * qb + 4  # causal 128-chunks for this q block
                            atT = p2t.tile([128, st_n * 512], F32R, tag="atT")
                            atv = atT[:].rearrange("p (kc f) -> p kc f", f=512)
                            for qs in range(4):
                                qt = 4 * qb + qs
                                nblk = qb + 1  # 512-wide k blocks
                                a_sb = p2a.tile([128, 2048], F32R, tag="a")
                                rs_all = p2s.tile([128, 4], F32, tag="rs")
                                for kb in range(nblk):
                                    ps_s = ps_s_p.tile([128, 512], F32, tag="s")
                                    nc.tensor.matmul(
                                        ps_s[:],
                                        r(qT_sb[:, qt * 512 + h * 128:qt * 512 + (h + 1) * 128]),
                                        r(kT_sb[:, kb * 512:(kb + 1) * 512]),
                                        start=True,
                                        stop=True,
                                    )
                                    if kb == qb:  # diagonal block: additive mask
                                        nc.vector.tensor_add(
                                            ps_s[:], ps_s[:], mask_sb[:, qs, :]
                                        )
                                    nc.scalar.activation(
                                        a_sb[:, kb * 512:(kb + 1) * 512],
                                        ps_s[:],
                                        mybir.ActivationFunctionType.Exp,
                                        scale=0.125,
                                        accum_out=rs_all[:, kb:kb + 1],
                                    )
                                tot = p2s.tile([128, 1], F32, tag="rtot")
                                nc.vector.reduce_sum(
                                    tot[:], rs_all[:, 0:nblk], axis=mybir.AxisListType.X
                                )
                                rinv = p2s.tile([128, 1], F32, tag="rinv")
                                nc.vector.reciprocal(rinv[:], tot[:])
                                for kb in range(nblk):
                                    nc.vector.tensor_scalar_mul(
                                        a_sb[:, kb * 512:(kb + 1) * 512],
                                        a_sb[:, kb * 512:(kb + 1) * 512],
                                        rinv[:],
                                    )
                                # transpose causal chunks kc <= qt into atT
                                for kg in range((qt + 4) // 4):
                                    cnt = min(4, qt + 1 - 4 * kg)
                                    ps_at = ps_at_p.tile([128, 512], F32, tag="at")
                                    for m in range(cnt):
                                        kc = 4 * kg + m
                                        nc.tensor.matmul(
                                            r(ps_at[:, m * 128:(m + 1) * 128]),
                                            r(a_sb[:, kc * 128:(kc + 1) * 128]),
                                            r(ident[:]),
                                            is_transpose=True,
                                            start=(m == 0),
                                            stop=(m == cnt - 1),
                                        )
                                    nc.vector.tensor_copy(
                                        atv[:, 4 * kg:4 * kg + cnt,
                                            qs * 128:(qs + 1) * 128],
                                        ps_at[:, 0:cnt * 128].rearrange(
                                            "p (a b) -> p a b", b=128
                                        ),
                                    )
                            # ---- AV: outT[d, q512] accumulated over k chunks ----
                            ps_av = ps_av_p.tile([128, 512], F32, tag="av")
                            for kc in range(nch):
                                # chunks past the diagonal have no attn mass for
                                # early q subtiles; skip those columns entirely
                                lo = max(0, kc - 4 * qb) * 128
                                nc.tensor.matmul(
                                    ps_av[:, lo:512],
                                    r(v_sb[:, kc * 2 * DKV:(kc + 1) * 2 * DKV]),
                                    r(atv[:, kc, lo:512]),
                                    start=(kc == 0),
                                    stop=(kc == nch - 1),
                                )
                            nc.vector.tensor_copy(
                                aoT_sb[p0:p0 + 64, hp2 * s_n + qb * 512:hp2 * s_n + (qb + 1) * 512],
                                ps_av[p0:p0 + 64, :],
                            )

                        # ---- phase 3 for this q block: output projection ----
                        for st in range(4 * qb, (4 * qb + 4) if 3 in phases else 4 * qb):
                            o_sb = p3o.tile([128, E], F32, tag="o")
                            for eb in range(E // 512):
                                ps_o = ps_o_p.tile([128, 512], F32, tag="po")
                                for c in range(2):
                                    nc.tensor.matmul(
                                        ps_o[:],
                                        r(aoT_sb[:, c * s_n + st * 128:c * s_n + (st + 1) * 128]),
                                        r(woT_sb[:, c, eb * 512:(eb + 1) * 512]),
                                        start=(c == 0),
                                        stop=(c == 1),
                                    )
                                nc.scalar.copy(o_sb[:, eb * 512:(eb + 1) * 512], ps_o[:])
                            nc.sync.dma_start(
                                out=out_d.ap()[st * 128:(st + 1) * 128, :], in_=o_sb[:]
                            )

    nc.compile()
    return nc


def make_tables(s_n=S):
    """Host-side RoPE tables and additive causal mask."""
    theta = (1.0 / (10000.0 ** (np.arange(0, HD, 2, dtype=np.float32) / HD))).astype(
        np.float32
    )
    freqs = np.arange(s_n, dtype=np.float32)[:, None] * theta[None, :]  # [s, 32]
    cos = np.cos(freqs).astype(np.float32)
    sin = np.sin(freqs).astype(np.float32)
    cosh = np.tile(cos, (1, DQK // HD))  # [s, 160]
    sinh = np.tile(sin, (1, DQK // HD))
    a = np.arange(512)
    maskadd = np.where(a[None, :] <= a[:, None], 0.0, MASK_NEG).astype(np.float32)
    return cosh, sinh, maskadd


def make_core_inputs(x2, wq, wk, wv, wo, core):
    """Per-core input dict (host-side sharding prep)."""
    cosh, sinh, maskadd = _TABLES
    i = core
    wq_i = wq[i * DQ:(i + 1) * DQ]
    wk_i = wk[i * DKV:(i + 1) * DKV]
    wv_i = wv[i * DKV:(i + 1) * DKV]
    wt = np.ascontiguousarray(np.concatenate([wq_i, wk_i, wv_i], axis=0).T)
    wot = np.ascontiguousarray(wo[:, i * DQ:(i + 1) * DQ].T)
    return {
        "x": x2,
        "wt": wt.astype(np.float32),
        "wot": wot.astype(np.float32),
        "cosh": cosh,
        "sinh": sinh,
        "maskadd": maskadd,
        "ident": np.eye(128, dtype=np.float32),
    }


_TABLES = make_tables()
_NC_CACHE = {}


def _get_nc(reps=1):
    key = ("nc", reps)
    if key not in _NC_CACHE:
        _NC_CACHE[key] = build_nc(reps=reps)
    return _NC_CACHE[key]


def kernel(x, wq, wk, wv, wo):
    x = np.asarray(x, dtype=np.float32)
    b, s_n, e = x.shape
    x2 = np.ascontiguousarray(x.reshape(s_n, e))
    in_maps = [
        make_core_inputs(x2, np.asarray(wq, np.float32), np.asarray(wk, np.float32),
                         np.asarray(wv, np.float32), np.asarray(wo, np.float32), i)
        for i in range(NCORES)
    ]
    res = run_bass_kernel_spmd(_get_nc(), in_maps, core_ids=list(range(NCORES)))
    out = np.zeros((s_n, e), dtype=np.float32)
    for rr in res.results:
        out += rr["out"]
    return out.reshape(b, s_n, e).astype(np.float32)



# revision 10
# speedup vs baseline: 1.0726x; 1.0726x over previous
"""Causal self-attention (GQA, RoPE) Trainium2 Bass kernel.

Full inputs in, full output out. Tensor-parallel over heads across 8
NeuronCores: core i computes q-heads 4i..4i+3 (kv head i) and a partial
output projection over its 256 attn-out features; the host sums the 8
partial outputs (the "all-reduce after output_proj" step).

v2 layout: scores are computed pre-transposed (S^T = k^T q, with k on
the psum partition dim), so the attention-probability transposes of the
v1 kernel disappear; the softmax denominator rides along the AV matmul
as an extra ones column of V, and the per-query normalization uses a
rank-1 (K=1) broadcast matmul plus one vector multiply. x arrives
host-pre-transposed, killing the on-chip x transposes.
"""

import numpy as np

import concourse.bacc as bacc
import concourse.mybir as mybir
import concourse.tile as tile
from concourse.bass_utils import run_bass_kernel_spmd

S = 2048          # sequence length
E = 2048          # embedding dim
H = 32            # query heads
KV = 8            # kv heads
HD = 64           # head dim
NCORES = 8
HC = H // NCORES  # query heads per core = 4
DQ = HC * HD      # per-core q proj width = 256
DKV = HD          # per-core kv proj width = 64
DQK = DQ + DKV    # roped span = 320
DW = DQ + 2 * DKV  # fused qkv proj width = 384
ST = S // 128     # 16 s-tiles of 128 rows
MASK_NEG = -1.0e4  # pre-scale additive mask (scaled: -1250 -> exp == 0)

F32 = mybir.dt.float32
F32R = mybir.dt.float32r


def r(ap):
    """Bitcast an AP to float32r so the PE runs fast-mode fp32 matmuls."""
    return ap.bitcast(F32R)


def build_nc(seq_tiles=ST, reps=1, phases=(1, 2, 3)):
    """Build + compile the per-core Bass program (identical on all cores)."""
    st_n = seq_tiles
    s_n = st_n * 128
    ng = st_n // 4    # 512-wide q groups

    nc = bacc.Bacc("TRN2", target_bir_lowering=False, debug=False)
    xt_d = nc.dram_tensor("xt", [E, s_n], F32R, kind="ExternalInput")
    wt_d = nc.dram_tensor("wt", [E, DW], F32R, kind="ExternalInput")
    wot_d = nc.dram_tensor("wot", [DQ, E], F32R, kind="ExternalInput")
    cos_d = nc.dram_tensor("cosh", [s_n, DQK // 2], F32, kind="ExternalInput")
    sin_d = nc.dram_tensor("sinh", [s_n, DQK // 2], F32, kind="ExternalInput")
    mask_d = nc.dram_tensor("maskt", [128, 4 * 512], F32, kind="ExternalInput")
    id_d = nc.dram_tensor("ident", [128, 128], F32R, kind="ExternalInput")
    one_d = nc.dram_tensor("ones", [128, 64], F32R, kind="ExternalInput")
    out_d = nc.dram_tensor("out", [s_n, E], F32, kind="ExternalOutput")

    with tile.TileContext(nc) as tc:
        for _rep in range(reps):
            with (
                tc.tile_pool(name="const", bufs=1) as constp,
                tc.tile_pool(name="store", bufs=1) as storep,
                tc.tile_pool(name="p1x", bufs=3) as p1x,
                tc.tile_pool(name="p1c", bufs=2) as p1c,
                tc.tile_pool(name="p1q", bufs=2) as p1q,
                tc.tile_pool(name="pexp", bufs=3) as pexp,
                tc.tile_pool(name="prv", bufs=2) as prv,
                tc.tile_pool(name="pbc", bufs=2) as pbc,
                tc.tile_pool(name="po", bufs=2) as po,
                # PSUM: 4 pools x 2 bufs x 1 bank = 8 banks exactly
                tc.tile_pool(name="psA", bufs=2, space="PSUM") as psA,
                tc.tile_pool(name="psS", bufs=2, space="PSUM") as psS,
                tc.tile_pool(name="psT", bufs=2, space="PSUM") as psT,
                tc.tile_pool(name="psV", bufs=2, space="PSUM") as psV,
            ):
                ident = constp.tile([128, 128], F32R)
                nc.sync.dma_start(out=ident[:], in_=id_d.ap()[:, :])
                ones_sb = constp.tile([128, 64], F32R)
                nc.sync.dma_start(out=ones_sb[:], in_=one_d.ap()[:, :])
                woT_sb = constp.tile([128, 2, E], F32R)
                nc.sync.dma_start(
                    out=woT_sb[:], in_=wot_d.ap().rearrange("(c p) e -> p c e", p=128)
                )
                mask_sb = constp.tile([128, 4, 512], F32)
                nc.sync.dma_start(
                    out=mask_sb[:], in_=mask_d.ap().rearrange("p (j k) -> p j k", j=4)
                )
                wT_sb = constp.tile([128, E // 128, DW], F32R)
                for j in range(E // 128):
                    nc.sync.dma_start(
                        out=wT_sb[:, j, :], in_=wt_d.ap()[j * 128:(j + 1) * 128, :]
                    )

                # persistent per-group stores
                qT = []   # [64, 4 heads x 512]: head h s-tile tt at h*512+tt*128
                kT = []   # [64, 512]
                vo = []   # [128, 4, 65]: per chunk [v(64) | 1]
                ao = []   # [128, 2, 512]: attn-out^T, head pair c on dim 1
                for g in range(ng):
                    qT.append(storep.tile([64, HC * 512], F32R, tag=f"qT{g}",
                                          name=f"qT{g}"))
                    kT.append(storep.tile([64, 512], F32R, tag=f"kT{g}",
                                          name=f"kT{g}"))
                    vo.append(storep.tile([128, 4, 65], F32R, tag=f"vo{g}",
                                          name=f"vo{g}"))
                    ao.append(storep.tile([128, 2, 512], F32R, tag=f"ao{g}",
                                          name=f"ao{g}"))
                for g in range(ng):
                    nc.vector.tensor_copy(
                        vo[g][:, :, 64:65],
                        ones_sb[:, 0:4].rearrange("p (a b) -> p a b", b=1),
                    )

                # ---------------- phase 1: qkv proj + rope ----------------
                def phase1_tile(t):
                    g, tt = divmod(t, 4)
                    xts = p1x.tile([128, E // 128, 128], F32R, tag="xts")
                    for j in range(E // 128):
                        nc.sync.dma_start(
                            out=xts[:, j, :],
                            in_=xt_d.ap()[j * 128:(j + 1) * 128,
                                          t * 128:(t + 1) * 128],
                        )
                    cs = p1c.tile([128, 2, DQK // 2], F32, tag="cs")
                    nc.sync.dma_start(
                        out=cs[:, 0, :], in_=cos_d.ap()[t * 128:(t + 1) * 128, :]
                    )
                    nc.sync.dma_start(
                        out=cs[:, 1, :], in_=sin_d.ap()[t * 128:(t + 1) * 128, :]
                    )
                    ps_qkv = psA.tile([128, 512], F32, tag="a")
                    for j in range(E // 128):
                        nc.tensor.matmul(
                            ps_qkv[:, 0:DW],
                            r(xts[:, j, :]),
                            r(wT_sb[:, j, :]),
                            start=(j == 0),
                            stop=(j == E // 128 - 1),
                        )
                    # rope on q+k jointly (320 cols)
                    pairs = DQK // 2
                    qk = p1q.tile([128, DQK], F32R, tag="qk")
                    se = ps_qkv[:, 0:DQK].rearrange("p (n two) -> p two n", two=2)
                    de = qk[:].rearrange("p (n two) -> p two n", two=2)
                    c_ap = cs[:, 0, :]
                    s_ap = cs[:, 1, :]
                    t1 = p1q.tile([128, pairs], F32, tag="t1")
                    t2 = p1q.tile([128, pairs], F32, tag="t2")
                    nc.vector.tensor_mul(t1[:], se[:, 0, :], c_ap)
                    nc.vector.tensor_mul(t2[:], se[:, 1, :], s_ap)
                    nc.vector.tensor_sub(de[:, 0, :], t1[:], t2[:])
                    t3 = p1q.tile([128, pairs], F32, tag="t3")
                    t4 = p1q.tile([128, pairs], F32, tag="t4")
                    nc.vector.tensor_mul(t3[:], se[:, 1, :], c_ap)
                    nc.vector.tensor_mul(t4[:], se[:, 0, :], s_ap)
                    nc.vector.tensor_add(de[:, 1, :], t3[:], t4[:])
                    # v chunk (cols 0:64 of the 65-col [v|1] slot)
                    nc.vector.tensor_copy(vo[g][:, tt, 0:64], ps_qkv[:, DQK:DW])
                    # transposes: q heads 0,1 + k, then q heads 2,3
                    tr1 = psT.tile([128, 512], F32, tag="t")
                    nc.tensor.matmul(r(tr1[0:64, 0:128]), r(qk[:, 0:64]),
                                     r(ident[:]), is_transpose=True,
                                     start=True, stop=False)
                    nc.tensor.matmul(r(tr1[0:64, 128:256]), r(qk[:, 64:128]),
                                     r(ident[:]), is_transpose=True,
                                     start=False, stop=False)
                    nc.tensor.matmul(r(tr1[0:64, 256:384]), r(qk[:, 256:320]),
                                     r(ident[:]), is_transpose=True,
                                     start=False, stop=True)
                    qTv = qT[g][:].rearrange("p (h c) -> p h c", c=512)
                    nc.vector.tensor_copy(
                        qTv[:, 0:2, tt * 128:(tt + 1) * 128],
                        tr1[0:64, 0:256].rearrange("p (h c) -> p h c", c=128),
                    )
                    nc.vector.tensor_copy(
                        kT[g][:, tt * 128:(tt + 1) * 128], tr1[0:64, 256:384]
                    )
                    tr2 = psT.tile([128, 512], F32, tag="t")
                    nc.tensor.matmul(r(tr2[0:64, 0:128]), r(qk[:, 128:192]),
                                     r(ident[:]), is_transpose=True,
                                     start=True, stop=False)
                    nc.tensor.matmul(r(tr2[0:64, 128:256]), r(qk[:, 192:256]),
                                     r(ident[:]), is_transpose=True,
                                     start=False, stop=True)
                    nc.vector.tensor_copy(
                        qTv[:, 2:4, tt * 128:(tt + 1) * 128],
                        tr2[0:64, 0:256].rearrange("p (h c) -> p h c", c=128),
                    )

                # ---------------- phase 2: attention (transposed) ----------
                def phase2_group(g):
                    for h in range(HC):
                        p0 = 64 * (h & 1)
                        hp2 = h >> 1
                        ps_av = psV.tile([128, 512], F32, tag="v")
                        nch = 4 * g + 4
                        for kc in range(nch):
                            kg, ko = divmod(kc, 4)
                            j = kc - 4 * g
                            lo = 0 if j < 0 else min(128 * j, 256)
                            ps_s = psS.tile([128, 512], F32, tag="s")
                            nc.tensor.matmul(
                                ps_s[:, lo:512],
                                r(kT[kg][:, ko * 128:(ko + 1) * 128]),
                                r(qT[g][:, h * 512 + lo:(h + 1) * 512]),
                                start=True, stop=True,
                            )
                            if j >= 0:
                                a, b = ((0, 128), (128, 256),
                                        (256, 384), (256, 512))[j]
                                nc.vector.tensor_add(
                                    ps_s[:, a:b], ps_s[:, a:b], mask_sb[:, j, a:b]
                                )
                            ex = pexp.tile([128, 512], F32R, tag="ex")
                            nc.scalar.activation(
                                ex[:, lo:512], ps_s[:, lo:512],
                                mybir.ActivationFunctionType.Exp, scale=0.125,
                            )
                            nc.tensor.matmul(
                                ps_av[0:65, lo:512],
                                vo[kg][:, ko, :],
                                ex[:, lo:512],
                                start=(kc == 0), stop=(kc == nch - 1),
                            )
                        # normalize: rows 0:64 are the head's out^T, row 64 is
                        # the softmax denominator. Broadcast 1/denom down 64
                        # rows with a K=1 matmul, stage to SBUF, multiply.
                        rv = prv.tile([128, 512], F32R, tag="rv")
                        with nc.allow_low_precision(reason="f32r bits == f32"):
                            nc.vector.reciprocal(rv[64:65, :], ps_av[64:65, :])
                        ps_b = psT.tile([128, 512], F32, tag="t")
                        nc.tensor.matmul(
                            ps_b[0:64, :],
                            ones_sb[64:65, 0:64],
                            rv[64:65, :],
                            start=True, stop=True,
                        )
                        bc = pbc.tile([128, 512], F32, tag="bc")
                        if p0 == 0:
                            nc.scalar.copy(bc[0:64, :], ps_b[0:64, :])
                        else:
                            nc.vector.tensor_copy(bc[64:128, :], ps_b[0:64, :])
                        nc.vector.tensor_mul(
                            ao[g][p0:p0 + 64, hp2, :],
                            ps_av[0:64, :],
                            bc[p0:p0 + 64, :],
                        )

                # ---------------- phase 3: output projection ----------------
                def phase3_tile(t):
                    g, tt = divmod(t, 4)
                    o_sb = po.tile([128, E], F32, tag="o")
                    for eb in range(4):
                        ps_o = psA.tile([128, 512], F32, tag="a")
                        for c in range(2):
                            nc.tensor.matmul(
                                ps_o[:],
                                r(ao[g][:, c, tt * 128:(tt + 1) * 128]),
                                r(woT_sb[:, c, eb * 512:(eb + 1) * 512]),
                                start=(c == 0), stop=(c == 1),
                            )
                        if eb < 2:
                            nc.scalar.copy(o_sb[:, eb * 512:(eb + 1) * 512], ps_o[:])
                        else:
                            nc.vector.tensor_copy(
                                o_sb[:, eb * 512:(eb + 1) * 512], ps_o[:]
                            )
                    nc.sync.dma_start(
                        out=out_d.ap()[t * 128:(t + 1) * 128, :], in_=o_sb[:]
                    )

                for g in range(ng):
                    if 1 in phases:
                        for tt in range(4):
                            phase1_tile(4 * g + tt)
                    if 2 in phases:
                        phase2_group(g)
                    if 3 in phases:
                        for tt in range(4):
                            phase3_tile(4 * g + tt)

    nc.compile()
    return nc


def make_tables(s_n=S):
    """Host-side RoPE tables and the S^T-layout additive causal masks."""
    theta = (1.0 / (10000.0 ** (np.arange(0, HD, 2, dtype=np.float32) / HD))).astype(
        np.float32
    )
    freqs = np.arange(s_n, dtype=np.float32)[:, None] * theta[None, :]  # [s, 32]
    cos = np.cos(freqs).astype(np.float32)
    sin = np.sin(freqs).astype(np.float32)
    cosh = np.tile(cos, (1, DQK // HD))  # [s, 160]
    sinh = np.tile(sin, (1, DQK // HD))
    # maskt[r, j*512 + c] = 0 iff q-col c >= 128*j + k-row r (unmasked)
    r_ = np.arange(128)[:, None]
    c_ = np.arange(512)[None, :]
    blocks = [
        np.where(c_ >= 128 * j + r_, 0.0, MASK_NEG).astype(np.float32)
        for j in range(4)
    ]
    maskt = np.ascontiguousarray(np.concatenate(blocks, axis=1))  # [128, 2048]
    return cosh, sinh, maskt


def make_core_inputs(x2, wq, wk, wv, wo, core):
    """Per-core input dict (host-side sharding prep)."""
    cosh, sinh, maskt = _TABLES
    i = core
    wq_i = wq[i * DQ:(i + 1) * DQ]
    wk_i = wk[i * DKV:(i + 1) * DKV]
    wv_i = wv[i * DKV:(i + 1) * DKV]
    wt = np.ascontiguousarray(np.concatenate([wq_i, wk_i, wv_i], axis=0).T)
    wot = np.ascontiguousarray(wo[:, i * DQ:(i + 1) * DQ].T)
    return {
        "xt": np.ascontiguousarray(np.asarray(x2, np.float32).T),
        "wt": wt.astype(np.float32),
        "wot": wot.astype(np.float32),
        "cosh": cosh,
        "sinh": sinh,
        "maskt": maskt,
        "ident": np.eye(128, dtype=np.float32),
        "ones": np.ones((128, 64), dtype=np.float32),
    }


_TABLES = make_tables()
_NC_CACHE = {}


def _get_nc(reps=1):
    key = ("nc", reps)
    if key not in _NC_CACHE:
        _NC_CACHE[key] = build_nc(reps=reps)
    return _NC_CACHE[key]


def kernel(x, wq, wk, wv, wo):
    x = np.asarray(x, dtype=np.float32)
    b, s_n, e = x.shape
    x2 = np.ascontiguousarray(x.reshape(s_n, e))
    in_maps = [
        make_core_inputs(x2, np.asarray(wq, np.float32), np.asarray(wk, np.float32),
                         np.asarray(wv, np.float32), np.asarray(wo, np.float32), i)
        for i in range(NCORES)
    ]
    res = run_bass_kernel_spmd(_get_nc(), in_maps, core_ids=list(range(NCORES)))
    out = np.zeros((s_n, e), dtype=np.float32)
    for rr in res.results:
        out += rr["out"]
    return out.reshape(b, s_n, e).astype(np.float32)
